# revision 111
# baseline (speedup 1.0000x reference)
"""CrossHeadAttention Trainium2 kernel (8-core SPMD, data+head parallel).

Reference computation (per batch b):
    k = x_enc @ Wk ; v = x_enc @ Wv ; q = x @ Wq        (bias-free linears)
    wei = softmax((q @ k^T) / sqrt(1024))  per head
    out = wei @ v                                        -> [B, T, H, D]

Sharding: 8 cores = 2 batches x 4 head-groups (4 heads each). Each core
receives x[b], x_enc[b] and the 256-column slice of Wq/Wk/Wv for its heads,
and produces out[b][:, :, hg*4:(hg+1)*4, :]. No cross-core communication.

Two tricks push both former bottlenecks (ScalarE exp stream ~134us, then
TensorE ~125us) down to ~100us:

1. EXP SPLIT. The exp stream (16.7M exps/core) is split between the
   ScalarE (native Exp) and the VectorE via a custom fused DVE op
   computing (1 + u(1 + u/2))^8 with u = x*scale/8 (8 ALU stages, ONE
   instruction per tile, 1 elem/cycle; max rel err vs exp ~0.6%). Each
   (st, tb, sw) slot is split into two per-head half-slots, each with its
   own 1-bank PSUM score tile ([128, 512] f32, 4-buffer ring) so both
   engines exp concurrently; a greedy virtual-time interleave assigns
   each of the 256 half-slots to the engine with less accumulated work.
   PSUM-touching housekeeping rides the exp engines (projection psum
   drains + first-pass merge copies on ScalarE, PV-merge adds on VectorE
   — GPSIMD cannot read PSUM); the SBUF-only finalize normalize-muls go
   to the otherwise-idle Pool engine (spread across all three in the
   tail). PV accumulates over two 8-st passes (xe chunks 2/3 projected
   mid-stream before their pass needs them).

2. SPLIT-PRECISION FP8 PROJECTIONS. q/k/v projections run as fp8e4m3
   DoubleRow matmuls (0.5 cyc/row, contraction pairs packed in dim1) on
   hi+lo split operands: x = x8 + xr8, W*32 = w8 + wr8 (the x32 puts W's
   N(0,1/1024) entries in fp8's normal range; q/k/v then carry x32, which
   cancels via exp-scale/1024 and a 32-valued softmax-sum ones column).
   Three DR terms (x8w8 + x8wr8 + xr8w8) give BETTER than bf16 accuracy
   (end-to-end 2.2e-3 vs 2.9e-3) at 3/4 the PE cost: projections drop
   from 41us to 31us. Host ships x/x_enc as packed hi/lo DR-layout fp8
   ([chunk, hi/lo, k, ctpair, 2, t], same total bytes as bf16); xe chunk
   0 additionally ships as two 256-token halves so the first scores' kT
   s-tiles ride a 1.5us DMA on the critical ramp.

  Attention, grouped by (st-range, t-block, head-pair):
    scores^T[s,t]: per head, one 512-wide bf16 matmul (kT slice
      stationary) into its own 1-bank psum ring slot
    p = exp(scores/32/1024) -> bf16
    PV transposed: p[s,128t] stationary, v_aug[s,65] moving -> acc[t,65];
      PV batches trail the exp stream by two slots.
  PV partials accumulate in two 1-bank psum tiles per group and merge into
  an SBUF accumulator at group end. Three UNEVEN st-passes {chunks 0+1},
  {2}, {3} defer the xe chunk builds to slots 64/96. Finalize: VectorE
  reciprocal of the sums, Pool per-partition scalar multiplies, two
  [128,2,128] f32 stores per (t-block, pair).

  Scheduling: projection work is emitted as "pieces" placed into specific
  exp slots, with chunk loads prefetched ~13 slots ahead and projections
  split into half bursts so no PE burst sits ahead of a score matmul.
"""

import os
from contextlib import ExitStack

import ml_dtypes
import numpy as np

import concourse.bacc as bacc
import concourse.tile as tile
from concourse import mybir
from concourse.bass_utils import run_bass_kernel_spmd

# Problem constants (hardcoded per spec)
B = 2
T = 2048          # query length
S = 2048          # key/value length
C = 1024          # n_embd
H = 16            # total heads
D = 64            # head size
N_CORES = 8
HG = H // (N_CORES // B)       # heads per core = 4
DCORE = HG * D                 # 256 projected dims per core
P = 128                        # partitions
NJ = 4                         # contraction ct-pairs (c = j*256 + i*128 + k)
NSW = 2                        # head-pair sweeps (dsl)
TB = 512                       # t-block width
NTB = T // TB                  # 4
ST = S // P                    # 16 s-tiles
NSP = 2                        # st halves (passes)
STQ = ST // NSP                # 8 s-tiles per pass (2 xe chunks)
NCH = 4                        # 512-row chunks per input tensor

F32 = mybir.dt.float32
BF16 = mybir.dt.bfloat16
FP8 = mybir.dt.float8e4
AF = mybir.ActivationFunctionType
DR = mybir.MatmulPerfMode.DoubleRow

WSCALE = 32.0                  # host W prescale into fp8 range
SCALE = float(C) ** -0.5       # 1/32
SCALE_Q = SCALE / (WSCALE * WSCALE)   # q,k each carry x32

# (w hi/lo, x hi/lo) term pairs: x8w8 + x8wr8 + xr8w8
TERMS = ((0, 0), (1, 0), (0, 1))

# Modeled per-op engine-busy costs (ns) used by the greedy exp-assignment
# balancer: each half-slot exp goes to whichever engine has less virtual
# accumulated work, counting the housekeeping that rides each engine
# (ACT: projection psum drains; DVE: PV merges + reciprocals — GPSIMD
# cannot read PSUM, so psum-touching housekeeping must ride these two).
ACT_EXP_NS = float(os.environ.get("K_AEXP", "612"))
DVE_EXP_NS = float(os.environ.get("K_DEXP", "658"))
ACT_DRAIN_NS = 612.0
DVE_MERGE_NS = float(os.environ.get("K_MRG", "396"))
DVE_RECIP_NS = 70.0

# Tuning knobs (env-overridable for schedule sweeps; defaults are tuned)
PEND_TRAIL = int(os.environ.get("K_PEND_TRAIL", "2"))
TWO_PASS = os.environ.get("K_TWO_PASS", "1") == "1"
P_BUFS = int(os.environ.get("K_PBUFS", "8"))
PSC_BUFS = int(os.environ.get("K_PSC", "4"))
PP_BUFS = int(os.environ.get("K_PPOOL", "2"))
# First slot of the paired-exp era: once the projection psum pool is dead,
# its banks (plus the split ring's) hold three 2-bank paired score tiles
# and each slot's two heads ride ONE exp instruction (less per-instruction
# access overhead on the exp engines, which pace the back of the kernel).
PAIR_START = int(os.environ.get("K_PAIR", "70"))
PAIR_MODE = os.environ.get("K_PMODE", "0") == "1"
TAIL_PRE_NS = float(os.environ.get("K_TAILPRE", "900"))
TAIL_PRELOAD = os.environ.get("K_TPL", "0") == "1"
LOOKAHEAD = int(os.environ.get("K_LA", "2"))
CLAMP_NS = float(os.environ.get("K_CLAMP", "2540"))
ACT_PAIR_NS = 1038.0
DVE_PAIR_NS = 1192.0
FRONT_SHARE = float(os.environ.get("K_FS", "-1"))   # <0 => greedy
BACK_SHARE = float(os.environ.get("K_BS", "0.55"))
MERGE_COPY_ACT = os.environ.get("K_MCA", "0") == "1"

# ---------------------------------------------------------------------------
# Custom DVE op: exp(x*scale) ~= (1 + u(1 + u/2))^8, u = x*scale/8.
# One 8-stage VectorE instruction per [128, 512] tile (1 elem/cycle/lane).
# Registered into concourse.dve_ops at import time (idempotent); the
# per-NEFF uop table is generated from OPS by name at compile-bir time.
# ---------------------------------------------------------------------------
EXP_NAME = "EXP_PSERIES_ANT"


def _register_exp_op():
    import concourse.dve_ops as dve_ops_mod
    from concourse.dve_spec import Spec, Src0, C0, C1, One, sq, lower, _has_src1
    from concourse.dve_uop import DveOpSpec

    for o in dve_ops_mod.OPS:
        if o.name == EXP_NAME:
            return o
    u = Src0 * C0
    body = u * C1 + One          # 1 + u/2   (C1 = 0.5)
    body = u * body + One        # 1 + u + u^2/2
    body = sq(sq(sq(body)))      # ^8

    def ref(in0, in1, s0, s1, imm2):
        uu = in0.astype(np.float32) * s0
        t = 1.0 + uu * (1.0 + s1 * uu)
        return (t * t) ** 4

    spec = Spec(body=body, reference=ref)
    row = dve_ops_mod._CUSTOM_DVE_ROW_BASE + len(dve_ops_mod.OPS)
    shas = {}
    for ver in ("v3", "v4"):
        uops = lower(spec, ver=ver)
        shas[ver] = DveOpSpec(
            name=EXP_NAME, opcode=row, uops=uops, rd1_en=_has_src1(spec)
        ).sha(ver)
    op = dve_ops_mod.DveOp(EXP_NAME, spec, subdim=False, uops_sha=shas)
    dve_ops_mod.OPS.append(op)
    dve_ops_mod.CUSTOM_DVE_SPECS[op.name] = spec
    dve_ops_mod._SUB_OPCODE_FOR_NAME[op.name] = row
    return op


_EXP_OP = _register_exp_op()


def _build_body(nc, tc, xpk, xepk, xe0, wq, wk, wv, out):
    with ExitStack() as ctx:
        consts = ctx.enter_context(tc.tile_pool(name="consts", bufs=1))
        big = ctx.enter_context(tc.tile_pool(name="big", bufs=1))
        xtp = ctx.enter_context(tc.tile_pool(name="xtp", bufs=5))
        pacc = ctx.enter_context(
            tc.tile_pool(name="pacc", bufs=2, space="PSUM"))
        ppool_sb = ctx.enter_context(
            tc.tile_pool(name="ppool_sb", bufs=P_BUFS))
        fin = ctx.enter_context(tc.tile_pool(name="fin", bufs=4))
        wpool = ctx.enter_context(tc.tile_pool(name="wpool", bufs=1))
        # ppool + psc are opened LAST (pools close in stack order) and live
        # only until the paired-exp era starts; closing them then releases
        # 6 psum banks for the paired score ring.
        front_psum = ExitStack()
        ppool = front_psum.enter_context(
            tc.tile_pool(name="ppool", bufs=2, space="PSUM"))
        psc = front_psum.enter_context(
            tc.tile_pool(name="psc", bufs=PSC_BUFS, space="PSUM"))
        psc2_box = [None]

        # prime the ScalarE exp table at t=0 so the ACT_TABLE_LOAD is off the
        # critical path of the first real exp
        dummy = consts.tile([1, 2], F32)
        nc.vector.memset(dummy, 0.0)
        nc.scalar.activation(out=dummy, in_=dummy, func=AF.Exp)

        # persistent activation-derived tensors (all carry the W x32 scale)
        kT = big.tile([P, NSW, S], BF16, tag="kT")      # [2h'*64d, dsl, s]
        qT = big.tile([P, NSW, T], BF16, tag="qT")
        v_sb = big.tile([P, ST, HG, D + 1], BF16, tag="v_sb")
        nc.vector.memset(v_sb[:, :, :, D], WSCALE)      # softmax-sum column
        acc_sb = big.tile([P, T // P, HG, D + 1], F32, tag="acc_sb")

        # weights: [P, hi/lo, j, i, d] fp8 per (name, dsl), one DMA each;
        # only the dsl=0 halves sit on the critical path.
        w_sbs = {}

        def load_w(name, wdram, dsl, split=False):
            """split=True ships hi before lo as separate DMAs so the ramp's
            hi-term matmuls start 2 weight-half transfers earlier."""
            def go():
                wsb = wpool.tile([P, 2, NJ, 2, P], FP8, tag=f"{name}{dsl}_sb",
                                 bufs=1, name=f"{name}{dsl}_sb")
                if split:
                    for hl in range(2):
                        nc.sync.dma_start(out=wsb[:, hl],
                                          in_=wdram[dsl, hl])
                else:
                    nc.sync.dma_start(
                        out=wsb,
                        in_=wdram[dsl].rearrange("hl p j i d -> p hl j i d"))
                w_sbs[(name, dsl)] = wsb
            return go

        # ------------------------------------------------------------------
        # projection pieces (closures). Row DMAs are split from the matmul
        # work so loads can be prefetched several slots ahead of the PE
        # stream that consumes them. All projection matmuls are fp8
        # DoubleRow over (hi/lo x hi/lo) split terms.
        # `state` carries live tiles per (src, chunk).
        # ------------------------------------------------------------------
        state = {}
        vtime = {"act": 0.0, "dve": 0.0}   # greedy exp-balancer clocks

        def drain_copy(out, in_, els):
            """psum->sbuf drain on ScalarE (statically: the activation
            engine absorbs these better than splitting them would — the
            VectorE queue must stay clear for merges at group boundaries)."""
            vtime["act"] += els * 0.833 + 185.0
            nc.scalar.copy(out=out, in_=in_)

        def chunk_load(src_dram, key, sch):
            """One contiguous DMA pulls a whole pre-packed hi/lo DR-layout
            chunk ([P, 2, j, i, 512] fp8, 8KB/partition) into SBUF."""
            def go():
                xt = xtp.tile([P, 2, NJ, 2, 512], FP8, tag="xch", name="xch",
                              bufs=6)
                nc.sync.dma_start(
                    out=xt,
                    in_=src_dram[sch].rearrange("s p j i t -> p s j i t"))
                state[key] = xt
            return go

        def chunk_load_part(src_dram, key, sch, s):
            """hi (s=0) or lo (s=1) part of a chunk as its own DMA into its
            own tile, so hi-term matmuls depend only on the hi transfer."""
            def go():
                if key not in state:
                    state[key] = ("HILO",
                                  xtp.tile([P, NJ, 2, 512], FP8, tag="xch2",
                                           name="xch2", bufs=2),
                                  xtp.tile([P, NJ, 2, 512], FP8, tag="xch2",
                                           name="xch2", bufs=2))
                nc.sync.dma_start(out=state[key][1 + s], in_=src_dram[sch, s])
            return go

        def xe0_half_load(half):
            """xe chunk 0 ships as two 256-token halves so the first
            scores' kT s-tiles ride a 1.5us DMA."""
            def go():
                xh = xtp.tile([P, 2, NJ, 2, 256], FP8, tag="xh0",
                              name="xh0", bufs=2)
                nc.sync.dma_start(
                    out=xh,
                    in_=xe0[half].rearrange("s p j i t -> p s j i t"))
                state[("xe0h", half)] = xh
                state[("xe", 0)] = "HALVES"
            return go

        def kq_dr(ps, w, xt, sl, out_sl, first, last):
            """12 DoubleRow matmuls (3 terms x 4 ct-pairs) accumulating
            W.T @ x[sl] into ps[:, out_sl]."""
            n = sl.stop - sl.start
            seq = [(whl, xs, j) for (whl, xs) in TERMS for j in range(NJ)]
            for idx, (whl, xs, j) in enumerate(seq):
                nc.tensor.matmul(
                    ps[:, out_sl.start:out_sl.start + n],
                    w[:, whl, j, :, :],
                    xt[:, xs, j, :, sl],
                    start=(first and idx == 0),
                    stop=(last and idx == len(seq) - 1),
                    perf_mode=DR,
                    skip_group_check=not (first and idx == 0))

        def kq_piece(wname, dst, key, sch, dsl, s4=None, part=None):
            """Project a chunk (or a token sub-range) of x/xe. part=0/1
            emits two bursts so long PE bursts never sit ahead of a score
            matmul."""
            if s4 is None:
                sl = slice(0, 512)
            elif isinstance(s4, tuple):
                sl = slice(s4[0] * P, s4[1] * P)
            else:
                sl = slice(s4 * P, (s4 + 1) * P)
            n = sl.stop - sl.start
            pskey = ("ps", wname, key, dsl, sl.start)

            def go():
                xt = state[key]
                halves = isinstance(xt, str)
                if part in (None, 0):
                    ps = ppool.tile([P, n], F32, tag="pp", bufs=PP_BUFS,
                                    name="ps")
                    state[pskey] = ps
                else:
                    ps = state.pop(pskey)
                w = w_sbs[(wname, dsl)]
                if halves:
                    # chunk-0 xe: tokens live in two half tiles; parts map
                    # to halves (each a full 12-matmul accumulation into
                    # its own region of ps, riding the pending-zero).
                    hlist = ((0, 1) if part is None else (part,))
                    for h in hlist:
                        h0, h1 = h * 256, (h + 1) * 256
                        if sl.start >= h1 or sl.stop <= h0:
                            continue
                        lo, hi = max(sl.start, h0), min(sl.stop, h1)
                        kq_dr(ps, w, state[("xe0h", h)],
                              slice(lo - h0, hi - h0),
                              slice(lo - sl.start, hi - sl.start),
                              first=(lo == sl.start), last=(hi == sl.stop))
                else:
                    hilo = isinstance(xt, tuple)
                    seq = [(whl, xs, j) for (whl, xs) in TERMS
                           for j in range(NJ)]
                    idxs = (range(12) if part is None
                            else range(part * 6, part * 6 + 6))
                    for idx in idxs:
                        whl, xs, j = seq[idx]
                        rhs = (xt[1 + xs][:, j, :, sl] if hilo
                               else xt[:, xs, j, :, sl])
                        nc.tensor.matmul(
                            ps, w[:, whl, j, :, :], rhs,
                            start=(idx == 0), stop=(idx == 11),
                            perf_mode=DR)
                if part in (None, 1):
                    drain_copy(
                        dst[:, dsl,
                            sch * 512 + sl.start:sch * 512 + sl.stop],
                        ps, n)
            return go

        def v_piece(key, sch, dsl, s4lo=0, s4hi=STQ // 2, part=None):
            """Project v for s-tiles [s4lo, s4hi) of a chunk into one psum
            bank (one start=True; later s-tiles ride the 2KB zero-region
            pending-zero) and drain with a single strided copy."""
            ns = s4hi - s4lo
            pskey = ("psv", key, dsl, s4lo)

            def go():
                xt = state[key]
                halves = isinstance(xt, str)
                if part in (None, 0):
                    ps = ppool.tile([P, ns, P], F32, tag="pp", bufs=PP_BUFS,
                                    name="psv")
                    state[pskey] = ps
                else:
                    ps = state.pop(pskey)
                w = w_sbs[("wv", dsl)]
                idxs = (range(ns) if part is None
                        else range(part * ns // 2, (part + 1) * ns // 2))
                for i in idxs:
                    s4 = s4lo + i
                    if halves:
                        src = state[("xe0h", s4 // 2)]
                        ssl = slice((s4 % 2) * P, (s4 % 2) * P + P)
                    else:
                        src = xt
                        ssl = slice(s4 * P, (s4 + 1) * P)
                    for ti, (whl, xs) in enumerate(TERMS):
                        for j in range(NJ):
                            nc.tensor.matmul(
                                ps[:, i, :],
                                src[:, xs, j, :, ssl],
                                w[:, whl, j, :, :],
                                start=(i == 0 and ti == 0 and j == 0),
                                stop=(s4 == s4hi - 1 and ti == len(TERMS) - 1
                                      and j == NJ - 1),
                                perf_mode=DR,
                                skip_group_check=True)
                if part in (None, 1):
                    drain_copy(
                        v_sb[:, sch * 4 + s4lo:sch * 4 + s4hi,
                             2 * dsl:2 * dsl + 2, 0:D],
                        ps.rearrange("p s (h d) -> p s h d", h=2), ns * P)
            return go

        # slot schedule: 128 slots; head pieces before slot 0, the rest
        # spread so chunk c is ready before the first group that needs it.
        NSLOT = NSP * NTB * NSW * STQ
        slot_sched = {i: [] for i in range(NSLOT)}

        def spread(pieces, lo, hi):
            n = len(pieces)
            lo, hi = max(lo, 0), max(hi, 1)
            span = max(hi - lo, 1)
            for i, pc in enumerate(pieces):
                slot_sched[min(lo + (i * span) // n, NSLOT - 1)].append(pc)

        # head: the critical chain is five DMAs (wq0, x^T chunk-0 hi, wk0,
        # xe^T chunk-0 half 0, x^T chunk-0 lo) plus two projection pieces.
        # x chunk 0 ships hi-part first so the two x8-terms of the q
        # projection overlap the xe half DMA.
        kx, kxe, kxe1 = ("x", 0), ("xe", 0), ("xe", 1)
        load_w("wq", wq, 0)()
        chunk_load_part(xpk, kx, 0, 0)()
        # the six hi-term q matmuls are emitted before the remaining head
        # loads so they depend only on wq + x0-hi and start mid-ramp
        kq_piece("wq", qT, kx, 0, 0, part=0)()
        load_w("wk", wk, 0)()
        xe0_half_load(0)()
        chunk_load_part(xpk, kx, 0, 1)()
        load_w("wv", wv, 0)()
        xe0_half_load(1)()
        chunk_load(xepk, kxe1, 1)()
        kq_piece("wq", qT, kx, 0, 0, part=1)()
        # the first scores need only s-tiles 0-1, which ride the first
        # half DMA; the s23 halves follow in the head stream
        kq_piece("wk", kT, kxe, 0, 0, s4=(0, 2))()
        v_piece(kxe, 0, 0, 0, 2)()
        kq_piece("wk", kT, kxe, 0, 0, s4=(2, 4))()
        v_piece(kxe, 0, 0, 2, 4)()

        # ramp era: chunk 1 and the dsl=1 halves in first-use order
        slot_sched[0] += [load_w("wq", wq, 1), load_w("wk", wk, 1),
                          load_w("wv", wv, 1),
                          kq_piece("wk", kT, kxe1, 1, 0, part=0)]
        slot_sched[1] += [kq_piece("wk", kT, kxe1, 1, 0, part=1),
                          v_piece(kxe1, 1, 0, part=0)]
        slot_sched[2] += [v_piece(kxe1, 1, 0, part=1)]
        slot_sched[3] += [kq_piece("wk", kT, kxe, 0, 1, part=0),
                          kq_piece("wk", kT, kxe, 0, 1, part=1)]
        slot_sched[4] += [kq_piece("wq", qT, kx, 0, 1, part=0),
                          kq_piece("wq", qT, kx, 0, 1, part=1),
                          v_piece(kxe, 0, 1, part=0)]
        slot_sched[5] += [v_piece(kxe, 0, 1, part=1),
                          kq_piece("wk", kT, kxe1, 1, 1, part=0)]
        slot_sched[6] += [kq_piece("wk", kT, kxe1, 1, 1, part=1),
                          v_piece(kxe1, 1, 1, part=0)]
        slot_sched[7] += [v_piece(kxe1, 1, 1, part=1)]

        # steady chunks: q(x-chunk tb) is first used at slot tb*16; the
        # pass-1/2 xe chunks at slots 64 / 96. Pieces spread over WIDE
        # windows so per-slot PE load stays near-uniform (narrow bursts
        # make those windows PE-bound while others sit exp-bound).
        qsh = int(os.environ.get("K_QSH", "2"))
        pfd = int(os.environ.get("K_PFD", "3"))
        for tb, (lo_, use) in ((1, (5 + qsh, 16)), (2, (18 + qsh, 32)),
                               (3, (34 + qsh, 48))):
            key = ("x", tb)
            spread([chunk_load(xpk, key, tb)], lo_ - pfd, lo_ - pfd + 1)
            spread([kq_piece("wq", qT, key, tb, d, part=pt)
                    for d in range(NSW) for pt in (0, 1)],
                   lo_, use - 1)
        xe2lo = int(os.environ.get("K_XE2LO", "36"))
        xe3lo = int(os.environ.get("K_XE3LO", "52"))
        xe_sched = (((2, (xe2lo, 64)), (3, (xe3lo, 68))) if TWO_PASS
                    else ((2, (36, 64)), (3, (66, 96))))
        for c, (lo_, use) in xe_sched:
            key = ("xe", c)
            spread([chunk_load(xepk, key, c)], lo_ - pfd + 1, lo_ - pfd + 2)
            spread([kq_piece("wk", kT, key, c, 0, part=0),
                    kq_piece("wk", kT, key, c, 0, part=1),
                    v_piece(key, c, 0, part=0),
                    v_piece(key, c, 0, part=1),
                    kq_piece("wk", kT, key, c, 1, part=0),
                    kq_piece("wk", kT, key, c, 1, part=1),
                    v_piece(key, c, 1, part=0),
                    v_piece(key, c, 1, part=1)],
                   lo_, use - 1)

        # Interleave of the 256 per-head exp half-slots between ScalarE
        # (native exp) and VectorE (custom poly op): greedy virtual-time by
        # default (each exp to the engine with less accumulated work, so the
        # split adapts per era), or fixed per-era Bresenham shares when
        # FRONT_SHARE >= 0.
        bres = {"n": 0, "a": 0}

        def emit_exp(p_tile, sc_tile, cur_slot, act_ns=ACT_EXP_NS,
                     dve_ns=DVE_EXP_NS):
            if FRONT_SHARE >= 0:
                share = FRONT_SHARE if cur_slot < 64 else BACK_SHARE
                bres["n"] += 1
                on_act = bres["a"] + 1 <= share * bres["n"]
                if on_act:
                    bres["a"] += 1
            else:
                on_act = vtime["act"] + act_ns <= vtime["dve"] + dve_ns
            if on_act:
                vtime["act"] += act_ns
                nc.scalar.activation(out=p_tile, in_=sc_tile, func=AF.Exp,
                                     scale=SCALE_Q)
            else:
                vtime["dve"] += dve_ns
                nc.vector._custom_dve(_EXP_OP, out=p_tile, in0=sc_tile,
                                      s0=SCALE_Q / 8.0, s1=0.5)

        # ------------------------------------------------------------------
        # attention: passes over uneven st ranges. Pass 0 covers xe chunks
        # 0-1 (built during the DMA-bound ramp); chunks 2 and 3 are only
        # pulled in at slots 64 / 96, so their projection pieces land in the
        # otherwise PE-lighter second half.
        # ------------------------------------------------------------------
        slot = 0
        passes = ([(0, 8), (8, 16)] if TWO_PASS
                  else [(0, 8), (8, 12), (12, 16)])
        glist = [(lo, hi, tb, sw) for (lo, hi) in passes
                 for tb in range(NTB) for sw in range(NSW)]
        lastv = {}
        seen = set()
        for gi, (lo, hi, tb, sw) in enumerate(glist):
            lastv[(tb, sw)] = gi
        for gi, (lo, hi, tb, sw) in enumerate(glist):
            last_g = gi == len(glist) - 1
            if last_g:
                # pre-charge the tail's DVE-only work so the balancer
                # leans the last exps onto ScalarE and both streams
                # drain together
                vtime["dve"] += TAIL_PRE_NS
            accs = [pacc.tile([P, 2, 2, D + 1], F32, tag="acc",
                              name=f"acc{a}") for a in range(2)]
            if last_g and TAIL_PRELOAD:
                # last group: preload its psum accumulators with the
                # pass-0 partials (copies run in earlier slack), PV
                # accumulates on top, and finalize reads PSUM directly —
                # the serial tail merges disappear
                for a in range(2):
                    src = acc_sb[:, tb * 4 + 2 * a: tb * 4 + 2 * a + 2,
                                 2 * sw:2 * sw + 2, :]
                    if a == 0:
                        vtime["act"] += DVE_MERGE_NS
                        nc.scalar.copy(out=accs[a], in_=src)
                    else:
                        vtime["dve"] += DVE_MERGE_NS
                        nc.vector.tensor_copy(out=accs[a], in_=src)
                first_pv = [False, False]
            else:
                first_pv = [True, True]

            def do_merge(a):
                dst = acc_sb[:, tb * 4 + 2 * a: tb * 4 + 2 * a + 2,
                             2 * sw:2 * sw + 2, :]
                # (vtime for merges is pre-charged 2 slots ahead in the
                # slot loop so the balancer has lookahead)
                if (tb, sw) not in seen:
                    if MERGE_COPY_ACT:
                        nc.scalar.copy(out=dst, in_=accs[a])
                    else:
                        nc.vector.tensor_copy(out=dst, in_=accs[a])
                else:
                    nc.vector.tensor_add(dst, accs[a], dst)

            def pv_batch(st, tail=False):
                ent = pend.pop(0)
                if ent[0] == "pair":
                    p2 = ent[1]
                    pts = (p2[:, 0], p2[:, 1])
                else:
                    pts = (ent[1], ent[2])
                for tt in range(TB // P):
                    a = tt // 2
                    for h2 in range(2):
                        nc.tensor.matmul(
                            accs[a][:, tt % 2, h2, :],
                            pts[h2][:, tt * P:(tt + 1) * P],
                            v_sb[:, st, 2 * sw + h2, :],
                            start=first_pv[a],
                            stop=(st == hi - 1 and tt % 2 == 1
                                  and h2 == 1),
                            skip_group_check=True)
                        first_pv[a] = False
                    # on the very last batch, merge each accumulator the
                    # moment its final PV is in and chase it with that
                    # half's normalize/store chain, pipelining the tail
                    if tail and tt % 2 == 1:
                        if TAIL_PRELOAD:
                            _finalize(nc, fin, acc_sb, out, tb, sw,
                                      half=tt // 2, on_act=True,
                                      psum_acc=accs[tt // 2])
                        else:
                            do_merge(tt // 2)
                            _finalize(nc, fin, acc_sb, out, tb, sw,
                                      half=tt // 2, on_act=True)

            pend = []
            for st in range(lo, hi):
                if st == hi - LOOKAHEAD and gi != len(glist) - 1:
                    # lookahead: charge this group's upcoming merges (and
                    # finalize reciprocals) before the last exps are
                    # assigned, so the engines drain together at the
                    # boundary
                    if (tb, sw) not in seen and MERGE_COPY_ACT:
                        vtime["act"] += 2 * DVE_MERGE_NS
                    else:
                        vtime["dve"] += 2 * DVE_MERGE_NS
                    if lastv[(tb, sw)] == gi:
                        vtime["dve"] += 2 * DVE_RECIP_NS
                # In the chunk-0/1 era, pieces PRODUCE the kT/qT/v this
                # very slot consumes, so they must precede it in the
                # in-order engine streams. In steady state pieces feed
                # later slots only and are emitted between the exp and the
                # trailing PV batch.
                if slot < 8:
                    for pc in slot_sched[slot]:
                        pc()
                if slot >= PAIR_START:
                    # late era: the projection psum pool is dead, so its
                    # banks widen the score ring. Two modes: 6 one-bank
                    # split tiles (3 slots of PE lookahead), or 3 two-bank
                    # paired tiles with ONE exp instruction per slot
                    # (less per-instruction overhead on the exp engines,
                    # which saturate in this era).
                    if psc2_box[0] is None:
                        front_psum.close()
                        psc2_box[0] = ctx.enter_context(
                            tc.tile_pool(name="psc2",
                                         bufs=(3 if PAIR_MODE else 6),
                                         space="PSUM"))
                    if PAIR_MODE:
                        sc2 = psc2_box[0].tile([P, 2, TB], F32, tag="scp",
                                               name="scp")
                        for h2 in range(2):
                            nc.tensor.matmul(
                                sc2[:, h2, :],
                                kT[h2 * D:(h2 + 1) * D, sw,
                                   st * P:(st + 1) * P],
                                qT[h2 * D:(h2 + 1) * D, sw,
                                   tb * TB:(tb + 1) * TB],
                                start=True, stop=True)
                        p2 = ppool_sb.tile([P, 2, TB], BF16, tag="p2",
                                           name="p2", bufs=4)
                        emit_exp(p2, sc2, slot, ACT_PAIR_NS, DVE_PAIR_NS)
                        pend.append(("pair", p2))
                    else:
                        ptiles = []
                        for h2 in range(2):
                            sc = psc2_box[0].tile([P, TB], F32, tag="sc2",
                                                  name="sc2")
                            nc.tensor.matmul(
                                sc,
                                kT[h2 * D:(h2 + 1) * D, sw,
                                   st * P:(st + 1) * P],
                                qT[h2 * D:(h2 + 1) * D, sw,
                                   tb * TB:(tb + 1) * TB],
                                start=True, stop=True)
                            p = ppool_sb.tile([P, TB], BF16, tag="p",
                                              name="p")
                            emit_exp(p, sc, slot)
                            ptiles.append(p)
                        pend.append(("split", ptiles[0], ptiles[1]))
                else:
                    ptiles = []
                    for h2 in range(2):
                        sc = psc.tile([P, TB], F32, tag="sc", name="sc")
                        nc.tensor.matmul(
                            sc,
                            kT[h2 * D:(h2 + 1) * D, sw, st * P:(st + 1) * P],
                            qT[h2 * D:(h2 + 1) * D, sw,
                               tb * TB:(tb + 1) * TB],
                            start=True, stop=True)
                        p = ppool_sb.tile([P, TB], BF16, tag="p", name="p")
                        emit_exp(p, sc, slot)
                        ptiles.append(p)
                    pend.append(("split", ptiles[0], ptiles[1]))
                if slot >= 8:
                    for pc in slot_sched[slot]:
                        pc()
                # forget old virtual-clock skew: in PE-bound eras both exp
                # engines idle together, so only ~1 slot of accumulated
                # imbalance is real
                m = max(vtime["act"], vtime["dve"]) - CLAMP_NS
                vtime["act"] = max(vtime["act"], m)
                vtime["dve"] = max(vtime["dve"], m)
                # PV batches trail PEND_TRAIL slots behind the exp stream
                if st - lo >= PEND_TRAIL:
                    pv_batch(st - PEND_TRAIL)
                if st == hi - 1:
                    for k in range(max(lo, hi - PEND_TRAIL), hi):
                        pv_batch(k, tail=(gi == len(glist) - 1
                                          and k == hi - 1))
                slot += 1
            # merge psum partials into the SBUF accumulator
            if gi != len(glist) - 1:
                for a in range(2):
                    do_merge(a)
            seen.add((tb, sw))
            if lastv[(tb, sw)] == gi and gi != len(glist) - 1:
                _finalize(nc, fin, acc_sb, out, tb, sw)
        if psc2_box[0] is None:
            front_psum.close()


def _finalize(nc, fin, acc_sb, out, tb, sw, half=None, on_act=False,
              psum_acc=None):
    """Normalize the finished heads of t-block tb and store. Two DMAs
    (2 t-tiles each) so the second store's DGE setup hides under the
    first's transfer; half=0/1 emits one accumulator-half's chain only
    (used to pipeline the very last group's tail). Reciprocal on VectorE,
    normalize-muls on Pool (on ScalarE for the tail, where it sits idle
    and Pool's serial launch overhead would stretch the ending)."""
    halves = (0, 1) if half is None else (half,)
    for h in halves:
        rcp = fin.tile([P, 2, 2], F32, tag="rcp", name="rcp")
        nc.vector.reciprocal(
            out=rcp,
            in_=(psum_acc[:, :, :, D] if psum_acc is not None else
                 acc_sb[:, tb * 4 + 2 * h:tb * 4 + 2 * h + 2,
                        2 * sw:2 * sw + 2, D]))
        ostage = fin.tile([P, 2, 2 * D], F32, tag="ost", name="ostage")
        for mi, (i, h2) in enumerate((i, h2) for i in range(2)
                                     for h2 in range(2)):
            tt4 = 2 * h + i
            dst = ostage[:, i, h2 * D:(h2 + 1) * D]
            src = (psum_acc[:, i, h2, 0:D] if psum_acc is not None else
                   acc_sb[:, tb * 4 + tt4, 2 * sw + h2, 0:D])
            scl = rcp[:, i, h2:h2 + 1]
            if psum_acc is not None:
                # psum source: only ScalarE/VectorE can read it
                if mi % 2 == 0:
                    nc.scalar.mul(out=dst, in_=src, mul=scl)
                else:
                    nc.vector.tensor_scalar_mul(out=dst, in0=src,
                                                scalar1=scl)
            elif not on_act:
                nc.gpsimd.tensor_scalar_mul(out=dst, in0=src, scalar1=scl)
            elif h == 1:
                # tail: the last store gates the kernel end; half-1's muls
                # run on the by-then-idle VectorE (94ns each) except one
                # on ScalarE for overlap
                if mi == 0:
                    nc.scalar.mul(out=dst, in_=src, mul=scl)
                else:
                    nc.vector.tensor_scalar_mul(out=dst, in0=src,
                                                scalar1=scl)
            elif mi == 0:
                # spread half-0's muls across ACT/Pool/DVE
                nc.scalar.mul(out=dst, in_=src, mul=scl)
            elif mi == 2:
                nc.vector.tensor_scalar_mul(out=dst, in0=src, scalar1=scl)
            else:
                nc.gpsimd.tensor_scalar_mul(out=dst, in0=src, scalar1=scl)
        t0 = (tb * 4 + h * 2) * P
        q = nc.scalar if (on_act and h == 0) else nc.sync
        q.dma_start(
            out=out[t0:t0 + 2 * P,
                    sw * 2 * D:(sw + 1) * 2 * D].rearrange(
                        "(tt p) c -> p tt c", p=P),
            in_=ostage)


def build_program():
    nc = bacc.Bacc("TRN2", target_bir_lowering=False, debug=False,
                   num_devices=N_CORES)

    # Host ships x/x_enc as packed hi/lo fp8 DR layouts (same bytes as
    # bf16) and W*32 hi/lo fp8; all transposes ride the DMA crossbar.
    xpk = nc.dram_tensor("x", [NCH, 2, P, NJ, 2, 512], FP8,
                         kind="ExternalInput").ap()
    xepk = nc.dram_tensor("xe", [NCH, 2, P, NJ, 2, 512], FP8,
                          kind="ExternalInput").ap()
    xe0 = nc.dram_tensor("xe0", [2, 2, P, NJ, 2, 256], FP8,
                         kind="ExternalInput").ap()
    wq = nc.dram_tensor("wq", [NSW, 2, P, NJ, 2, P], FP8,
                        kind="ExternalInput").ap()
    wk = nc.dram_tensor("wk", [NSW, 2, P, NJ, 2, P], FP8,
                        kind="ExternalInput").ap()
    wv = nc.dram_tensor("wv", [NSW, 2, P, NJ, 2, P], FP8,
                        kind="ExternalInput").ap()
    out = nc.dram_tensor("out", [T, DCORE], F32, kind="ExternalOutput").ap()

    with tile.TileContext(nc) as tc:
        _build_body(nc, tc, xpk, xepk, xe0, wq, wk, wv, out)
    nc.compile()
    return nc


_NC_CACHE = None


def _get_program():
    global _NC_CACHE
    if _NC_CACHE is None:
        _NC_CACHE = build_program()
    return _NC_CACHE


_F8 = ml_dtypes.float8_e4m3


def _split8(a):
    hi = a.astype(_F8)
    lo = (a - hi.astype(np.float32)).astype(_F8)
    return hi, lo


def _pack_x(xT):
    """[C, T] f32 -> [NCH, 2, P, NJ, 2, 512] fp8 hi/lo, c = j*256+i*128+k."""
    o = np.empty((NCH, 2, P, NJ, 2, 512), dtype=_F8)
    for cch in range(NCH):
        xc = xT[:, cch * 512:(cch + 1) * 512]
        hi, lo = _split8(xc)
        for t, a in ((0, hi), (1, lo)):
            o[cch, t] = a.reshape(NJ, 2, P, 512).transpose(2, 0, 1, 3)
    return np.ascontiguousarray(o)


def _pack_xe0(xeT):
    """First 512 cols of xe^T -> [2 half, 2, P, NJ, 2, 256] fp8."""
    o = np.empty((2, 2, P, NJ, 2, 256), dtype=_F8)
    for h in range(2):
        xc = xeT[:, h * 256:(h + 1) * 256]
        hi, lo = _split8(xc)
        for t, a in ((0, hi), (1, lo)):
            o[h, t] = a.reshape(NJ, 2, P, 256).transpose(2, 0, 1, 3)
    return np.ascontiguousarray(o)


def _pack_w(w):
    """[1024, 256] f32 -> [NSW, 2, P, NJ, 2, P] fp8 of W*32 hi/lo."""
    w = w * WSCALE
    o = np.empty((NSW, 2, P, NJ, 2, P), dtype=_F8)
    for dsl in range(NSW):
        wd = w.reshape(C, NSW, P)[:, dsl, :]          # cols = dsl*128 + d
        hi, lo = _split8(wd)
        for t, a in ((0, hi), (1, lo)):
            o[dsl, t] = a.reshape(NJ, 2, P, P).transpose(2, 0, 1, 3)
    return np.ascontiguousarray(o)


def kernel(x_enc, x, Wk, Wq, Wv):
    x_enc = np.asarray(x_enc, dtype=np.float32)
    x = np.asarray(x, dtype=np.float32)
    Wk = np.asarray(Wk, dtype=np.float32)
    Wq = np.asarray(Wq, dtype=np.float32)
    Wv = np.asarray(Wv, dtype=np.float32)

    nc = _get_program()
    in_maps = []
    for core in range(N_CORES):
        b, hg = divmod(core, N_CORES // B)
        csl = slice(hg * DCORE, (hg + 1) * DCORE)
        xT = np.ascontiguousarray(x[b].T)
        xeT = np.ascontiguousarray(x_enc[b].T)
        in_maps.append({
            "x": _pack_x(xT),
            "xe": _pack_x(xeT),
            "xe0": _pack_xe0(xeT),
            "wq": _pack_w(Wq[:, csl]),
            "wk": _pack_w(Wk[:, csl]),
            "wv": _pack_w(Wv[:, csl]),
        })
    res = run_bass_kernel_spmd(nc, in_maps, list(range(N_CORES)))

    full = np.empty((B, T, H, D), dtype=np.float32)
    for core in range(N_CORES):
        b, hg = divmod(core, N_CORES // B)
        o = res.results[core]["out"].reshape(T, HG, D)
        full[b, :, hg * HG:(hg + 1) * HG, :] = o
    return full


# revision 112
# speedup vs baseline: 1.0007x; 1.0007x over previous
"""CrossHeadAttention Trainium2 kernel (8-core SPMD, data+head parallel).

Reference computation (per batch b):
    k = x_enc @ Wk ; v = x_enc @ Wv ; q = x @ Wq        (bias-free linears)
    wei = softmax((q @ k^T) / sqrt(1024))  per head
    out = wei @ v                                        -> [B, T, H, D]

Sharding: 8 cores = 2 batches x 4 head-groups (4 heads each). Each core
receives x[b], x_enc[b] and the 256-column slice of Wq/Wk/Wv for its heads,
and produces out[b][:, :, hg*4:(hg+1)*4, :]. No cross-core communication.

Two tricks push both former bottlenecks (ScalarE exp stream ~134us, then
TensorE ~125us) down to ~100us:

1. EXP SPLIT. The exp stream (16.7M exps/core) is split between the
   ScalarE (native Exp) and the VectorE via a custom fused DVE op
   computing (1 + u(1 + u/2))^8 with u = x*scale/8 (8 ALU stages, ONE
   instruction per tile, 1 elem/cycle; max rel err vs exp ~0.6%). Each
   (st, tb, sw) slot is split into two per-head half-slots, each with its
   own 1-bank PSUM score tile ([128, 512] f32, 4-buffer ring) so both
   engines exp concurrently; a greedy virtual-time interleave assigns
   each of the 256 half-slots to the engine with less accumulated work.
   PSUM-touching housekeeping rides the exp engines (projection psum
   drains + first-pass merge copies on ScalarE, PV-merge adds on VectorE
   — GPSIMD cannot read PSUM); the SBUF-only finalize normalize-muls go
   to the otherwise-idle Pool engine (spread across all three in the
   tail). PV accumulates over two 8-st passes (xe chunks 2/3 projected
   mid-stream before their pass needs them).

2. SPLIT-PRECISION FP8 PROJECTIONS. q/k/v projections run as fp8e4m3
   DoubleRow matmuls (0.5 cyc/row, contraction pairs packed in dim1) on
   hi+lo split operands: x = x8 + xr8, W*32 = w8 + wr8 (the x32 puts W's
   N(0,1/1024) entries in fp8's normal range; q/k/v then carry x32, which
   cancels via exp-scale/1024 and a 32-valued softmax-sum ones column).
   Three DR terms (x8w8 + x8wr8 + xr8w8) give BETTER than bf16 accuracy
   (end-to-end 2.2e-3 vs 2.9e-3) at 3/4 the PE cost: projections drop
   from 41us to 31us. Host ships x/x_enc as packed hi/lo DR-layout fp8
   ([chunk, hi/lo, k, ctpair, 2, t], same total bytes as bf16); xe chunk
   0 additionally ships as two 256-token halves so the first scores' kT
   s-tiles ride a 1.5us DMA on the critical ramp.

  Attention, grouped by (st-range, t-block, head-pair):
    scores^T[s,t]: per head, one 512-wide bf16 matmul (kT slice
      stationary) into its own 1-bank psum ring slot
    p = exp(scores/32/1024) -> bf16
    PV transposed: p[s,128t] stationary, v_aug[s,65] moving -> acc[t,65];
      PV batches trail the exp stream by two slots.
  PV partials accumulate in two 1-bank psum tiles per group and merge into
  an SBUF accumulator at group end. Three UNEVEN st-passes {chunks 0+1},
  {2}, {3} defer the xe chunk builds to slots 64/96. Finalize: VectorE
  reciprocal of the sums, Pool per-partition scalar multiplies, two
  [128,2,128] f32 stores per (t-block, pair).

  Scheduling: projection work is emitted as "pieces" placed into specific
  exp slots, with chunk loads prefetched ~13 slots ahead and projections
  split into half bursts so no PE burst sits ahead of a score matmul.
"""

import os
from contextlib import ExitStack

import ml_dtypes
import numpy as np

import concourse.bacc as bacc
import concourse.tile as tile
from concourse import mybir
from concourse.bass_utils import run_bass_kernel_spmd

# Problem constants (hardcoded per spec)
B = 2
T = 2048          # query length
S = 2048          # key/value length
C = 1024          # n_embd
H = 16            # total heads
D = 64            # head size
N_CORES = 8
HG = H // (N_CORES // B)       # heads per core = 4
DCORE = HG * D                 # 256 projected dims per core
P = 128                        # partitions
NJ = 4                         # contraction ct-pairs (c = j*256 + i*128 + k)
NSW = 2                        # head-pair sweeps (dsl)
TB = 512                       # t-block width
NTB = T // TB                  # 4
ST = S // P                    # 16 s-tiles
NSP = 2                        # st halves (passes)
STQ = ST // NSP                # 8 s-tiles per pass (2 xe chunks)
NCH = 4                        # 512-row chunks per input tensor

F32 = mybir.dt.float32
BF16 = mybir.dt.bfloat16
FP8 = mybir.dt.float8e4
AF = mybir.ActivationFunctionType
DR = mybir.MatmulPerfMode.DoubleRow

WSCALE = 32.0                  # host W prescale into fp8 range
SCALE = float(C) ** -0.5       # 1/32
SCALE_Q = SCALE / (WSCALE * WSCALE)   # q,k each carry x32

# (w hi/lo, x hi/lo) term pairs: x8w8 + x8wr8 + xr8w8
TERMS = ((0, 0), (1, 0), (0, 1))

# Modeled per-op engine-busy costs (ns) used by the greedy exp-assignment
# balancer: each half-slot exp goes to whichever engine has less virtual
# accumulated work, counting the housekeeping that rides each engine
# (ACT: projection psum drains; DVE: PV merges + reciprocals — GPSIMD
# cannot read PSUM, so psum-touching housekeeping must ride these two).
ACT_EXP_NS = float(os.environ.get("K_AEXP", "612"))
DVE_EXP_NS = float(os.environ.get("K_DEXP", "659"))
ACT_DRAIN_NS = 612.0
DVE_MERGE_NS = float(os.environ.get("K_MRG", "396"))
DVE_RECIP_NS = 70.0

# Tuning knobs (env-overridable for schedule sweeps; defaults are tuned)
PEND_TRAIL = int(os.environ.get("K_PEND_TRAIL", "2"))
TWO_PASS = os.environ.get("K_TWO_PASS", "1") == "1"
P_BUFS = int(os.environ.get("K_PBUFS", "8"))
PSC_BUFS = int(os.environ.get("K_PSC", "4"))
PP_BUFS = int(os.environ.get("K_PPOOL", "2"))
# First slot of the paired-exp era: once the projection psum pool is dead,
# its banks (plus the split ring's) hold three 2-bank paired score tiles
# and each slot's two heads ride ONE exp instruction (less per-instruction
# access overhead on the exp engines, which pace the back of the kernel).
PAIR_START = int(os.environ.get("K_PAIR", "70"))
PAIR_MODE = os.environ.get("K_PMODE", "0") == "1"
TAIL_PRE_NS = float(os.environ.get("K_TAILPRE", "900"))
TAIL_PRELOAD = os.environ.get("K_TPL", "0") == "1"
LOOKAHEAD = int(os.environ.get("K_LA", "2"))
CLAMP_NS = float(os.environ.get("K_CLAMP", "2540"))
ACT_PAIR_NS = 1038.0
DVE_PAIR_NS = 1192.0
FRONT_SHARE = float(os.environ.get("K_FS", "-1"))   # <0 => greedy
BACK_SHARE = float(os.environ.get("K_BS", "0.55"))
MERGE_COPY_ACT = os.environ.get("K_MCA", "0") == "1"

# ---------------------------------------------------------------------------
# Custom DVE op: exp(x*scale) ~= (1 + u(1 + u/2))^8, u = x*scale/8.
# One 8-stage VectorE instruction per [128, 512] tile (1 elem/cycle/lane).
# Registered into concourse.dve_ops at import time (idempotent); the
# per-NEFF uop table is generated from OPS by name at compile-bir time.
# ---------------------------------------------------------------------------
EXP_NAME = "EXP_PSERIES_ANT"


def _register_exp_op():
    import concourse.dve_ops as dve_ops_mod
    from concourse.dve_spec import Spec, Src0, C0, C1, One, sq, lower, _has_src1
    from concourse.dve_uop import DveOpSpec

    for o in dve_ops_mod.OPS:
        if o.name == EXP_NAME:
            return o
    u = Src0 * C0
    body = u * C1 + One          # 1 + u/2   (C1 = 0.5)
    body = u * body + One        # 1 + u + u^2/2
    body = sq(sq(sq(body)))      # ^8

    def ref(in0, in1, s0, s1, imm2):
        uu = in0.astype(np.float32) * s0
        t = 1.0 + uu * (1.0 + s1 * uu)
        return (t * t) ** 4

    spec = Spec(body=body, reference=ref)
    row = dve_ops_mod._CUSTOM_DVE_ROW_BASE + len(dve_ops_mod.OPS)
    shas = {}
    for ver in ("v3", "v4"):
        uops = lower(spec, ver=ver)
        shas[ver] = DveOpSpec(
            name=EXP_NAME, opcode=row, uops=uops, rd1_en=_has_src1(spec)
        ).sha(ver)
    op = dve_ops_mod.DveOp(EXP_NAME, spec, subdim=False, uops_sha=shas)
    dve_ops_mod.OPS.append(op)
    dve_ops_mod.CUSTOM_DVE_SPECS[op.name] = spec
    dve_ops_mod._SUB_OPCODE_FOR_NAME[op.name] = row
    return op


_EXP_OP = _register_exp_op()


def _build_body(nc, tc, xpk, xepk, xe0, wq, wk, wv, out):
    with ExitStack() as ctx:
        consts = ctx.enter_context(tc.tile_pool(name="consts", bufs=1))
        big = ctx.enter_context(tc.tile_pool(name="big", bufs=1))
        xtp = ctx.enter_context(tc.tile_pool(name="xtp", bufs=5))
        pacc = ctx.enter_context(
            tc.tile_pool(name="pacc", bufs=2, space="PSUM"))
        ppool_sb = ctx.enter_context(
            tc.tile_pool(name="ppool_sb", bufs=P_BUFS))
        fin = ctx.enter_context(tc.tile_pool(name="fin", bufs=4))
        wpool = ctx.enter_context(tc.tile_pool(name="wpool", bufs=1))
        # ppool + psc are opened LAST (pools close in stack order) and live
        # only until the paired-exp era starts; closing them then releases
        # 6 psum banks for the paired score ring.
        front_psum = ExitStack()
        ppool = front_psum.enter_context(
            tc.tile_pool(name="ppool", bufs=2, space="PSUM"))
        psc = front_psum.enter_context(
            tc.tile_pool(name="psc", bufs=PSC_BUFS, space="PSUM"))
        psc2_box = [None]

        # prime the ScalarE exp table at t=0 so the ACT_TABLE_LOAD is off the
        # critical path of the first real exp
        dummy = consts.tile([1, 2], F32)
        nc.vector.memset(dummy, 0.0)
        nc.scalar.activation(out=dummy, in_=dummy, func=AF.Exp)

        # persistent activation-derived tensors (all carry the W x32 scale)
        kT = big.tile([P, NSW, S], BF16, tag="kT")      # [2h'*64d, dsl, s]
        qT = big.tile([P, NSW, T], BF16, tag="qT")
        v_sb = big.tile([P, ST, HG, D + 1], BF16, tag="v_sb")
        nc.vector.memset(v_sb[:, :, :, D], WSCALE)      # softmax-sum column
        acc_sb = big.tile([P, T // P, HG, D + 1], F32, tag="acc_sb")

        # weights: [P, hi/lo, j, i, d] fp8 per (name, dsl), one DMA each;
        # only the dsl=0 halves sit on the critical path.
        w_sbs = {}

        def load_w(name, wdram, dsl, split=False):
            """split=True ships hi before lo as separate DMAs so the ramp's
            hi-term matmuls start 2 weight-half transfers earlier."""
            def go():
                wsb = wpool.tile([P, 2, NJ, 2, P], FP8, tag=f"{name}{dsl}_sb",
                                 bufs=1, name=f"{name}{dsl}_sb")
                if split:
                    for hl in range(2):
                        nc.sync.dma_start(out=wsb[:, hl],
                                          in_=wdram[dsl, hl])
                else:
                    nc.sync.dma_start(
                        out=wsb,
                        in_=wdram[dsl].rearrange("hl p j i d -> p hl j i d"))
                w_sbs[(name, dsl)] = wsb
            return go

        # ------------------------------------------------------------------
        # projection pieces (closures). Row DMAs are split from the matmul
        # work so loads can be prefetched several slots ahead of the PE
        # stream that consumes them. All projection matmuls are fp8
        # DoubleRow over (hi/lo x hi/lo) split terms.
        # `state` carries live tiles per (src, chunk).
        # ------------------------------------------------------------------
        state = {}
        vtime = {"act": 0.0, "dve": 0.0}   # greedy exp-balancer clocks

        def drain_copy(out, in_, els):
            """psum->sbuf drain on ScalarE (statically: the activation
            engine absorbs these better than splitting them would — the
            VectorE queue must stay clear for merges at group boundaries)."""
            vtime["act"] += els * 0.833 + 185.0
            nc.scalar.copy(out=out, in_=in_)

        def chunk_load(src_dram, key, sch):
            """One contiguous DMA pulls a whole pre-packed hi/lo DR-layout
            chunk ([P, 2, j, i, 512] fp8, 8KB/partition) into SBUF."""
            def go():
                xt = xtp.tile([P, 2, NJ, 2, 512], FP8, tag="xch", name="xch",
                              bufs=6)
                nc.sync.dma_start(
                    out=xt,
                    in_=src_dram[sch].rearrange("s p j i t -> p s j i t"))
                state[key] = xt
            return go

        def chunk_load_part(src_dram, key, sch, s):
            """hi (s=0) or lo (s=1) part of a chunk as its own DMA into its
            own tile, so hi-term matmuls depend only on the hi transfer."""
            def go():
                if key not in state:
                    state[key] = ("HILO",
                                  xtp.tile([P, NJ, 2, 512], FP8, tag="xch2",
                                           name="xch2", bufs=2),
                                  xtp.tile([P, NJ, 2, 512], FP8, tag="xch2",
                                           name="xch2", bufs=2))
                nc.sync.dma_start(out=state[key][1 + s], in_=src_dram[sch, s])
            return go

        def xe0_half_load(half):
            """xe chunk 0 ships as two 256-token halves so the first
            scores' kT s-tiles ride a 1.5us DMA."""
            def go():
                xh = xtp.tile([P, 2, NJ, 2, 256], FP8, tag="xh0",
                              name="xh0", bufs=2)
                nc.sync.dma_start(
                    out=xh,
                    in_=xe0[half].rearrange("s p j i t -> p s j i t"))
                state[("xe0h", half)] = xh
                state[("xe", 0)] = "HALVES"
            return go

        def kq_dr(ps, w, xt, sl, out_sl, first, last):
            """12 DoubleRow matmuls (3 terms x 4 ct-pairs) accumulating
            W.T @ x[sl] into ps[:, out_sl]."""
            n = sl.stop - sl.start
            seq = [(whl, xs, j) for (whl, xs) in TERMS for j in range(NJ)]
            for idx, (whl, xs, j) in enumerate(seq):
                nc.tensor.matmul(
                    ps[:, out_sl.start:out_sl.start + n],
                    w[:, whl, j, :, :],
                    xt[:, xs, j, :, sl],
                    start=(first and idx == 0),
                    stop=(last and idx == len(seq) - 1),
                    perf_mode=DR,
                    skip_group_check=not (first and idx == 0))

        def kq_piece(wname, dst, key, sch, dsl, s4=None, part=None):
            """Project a chunk (or a token sub-range) of x/xe. part=0/1
            emits two bursts so long PE bursts never sit ahead of a score
            matmul."""
            if s4 is None:
                sl = slice(0, 512)
            elif isinstance(s4, tuple):
                sl = slice(s4[0] * P, s4[1] * P)
            else:
                sl = slice(s4 * P, (s4 + 1) * P)
            n = sl.stop - sl.start
            pskey = ("ps", wname, key, dsl, sl.start)

            def go():
                xt = state[key]
                halves = isinstance(xt, str)
                if part in (None, 0):
                    ps = ppool.tile([P, n], F32, tag="pp", bufs=PP_BUFS,
                                    name="ps")
                    state[pskey] = ps
                else:
                    ps = state.pop(pskey)
                w = w_sbs[(wname, dsl)]
                if halves:
                    # chunk-0 xe: tokens live in two half tiles; parts map
                    # to halves (each a full 12-matmul accumulation into
                    # its own region of ps, riding the pending-zero).
                    hlist = ((0, 1) if part is None else (part,))
                    for h in hlist:
                        h0, h1 = h * 256, (h + 1) * 256
                        if sl.start >= h1 or sl.stop <= h0:
                            continue
                        lo, hi = max(sl.start, h0), min(sl.stop, h1)
                        kq_dr(ps, w, state[("xe0h", h)],
                              slice(lo - h0, hi - h0),
                              slice(lo - sl.start, hi - sl.start),
                              first=(lo == sl.start), last=(hi == sl.stop))
                else:
                    hilo = isinstance(xt, tuple)
                    seq = [(whl, xs, j) for (whl, xs) in TERMS
                           for j in range(NJ)]
                    idxs = (range(12) if part is None
                            else range(part * 6, part * 6 + 6))
                    for idx in idxs:
                        whl, xs, j = seq[idx]
                        rhs = (xt[1 + xs][:, j, :, sl] if hilo
                               else xt[:, xs, j, :, sl])
                        nc.tensor.matmul(
                            ps, w[:, whl, j, :, :], rhs,
                            start=(idx == 0), stop=(idx == 11),
                            perf_mode=DR)
                if part in (None, 1):
                    drain_copy(
                        dst[:, dsl,
                            sch * 512 + sl.start:sch * 512 + sl.stop],
                        ps, n)
            return go

        def v_piece(key, sch, dsl, s4lo=0, s4hi=STQ // 2, part=None):
            """Project v for s-tiles [s4lo, s4hi) of a chunk into one psum
            bank (one start=True; later s-tiles ride the 2KB zero-region
            pending-zero) and drain with a single strided copy."""
            ns = s4hi - s4lo
            pskey = ("psv", key, dsl, s4lo)

            def go():
                xt = state[key]
                halves = isinstance(xt, str)
                if part in (None, 0):
                    ps = ppool.tile([P, ns, P], F32, tag="pp", bufs=PP_BUFS,
                                    name="psv")
                    state[pskey] = ps
                else:
                    ps = state.pop(pskey)
                w = w_sbs[("wv", dsl)]
                idxs = (range(ns) if part is None
                        else range(part * ns // 2, (part + 1) * ns // 2))
                for i in idxs:
                    s4 = s4lo + i
                    if halves:
                        src = state[("xe0h", s4 // 2)]
                        ssl = slice((s4 % 2) * P, (s4 % 2) * P + P)
                    else:
                        src = xt
                        ssl = slice(s4 * P, (s4 + 1) * P)
                    for ti, (whl, xs) in enumerate(TERMS):
                        for j in range(NJ):
                            nc.tensor.matmul(
                                ps[:, i, :],
                                src[:, xs, j, :, ssl],
                                w[:, whl, j, :, :],
                                start=(i == 0 and ti == 0 and j == 0),
                                stop=(s4 == s4hi - 1 and ti == len(TERMS) - 1
                                      and j == NJ - 1),
                                perf_mode=DR,
                                skip_group_check=True)
                if part in (None, 1):
                    drain_copy(
                        v_sb[:, sch * 4 + s4lo:sch * 4 + s4hi,
                             2 * dsl:2 * dsl + 2, 0:D],
                        ps.rearrange("p s (h d) -> p s h d", h=2), ns * P)
            return go

        # slot schedule: 128 slots; head pieces before slot 0, the rest
        # spread so chunk c is ready before the first group that needs it.
        NSLOT = NSP * NTB * NSW * STQ
        slot_sched = {i: [] for i in range(NSLOT)}

        def spread(pieces, lo, hi):
            n = len(pieces)
            lo, hi = max(lo, 0), max(hi, 1)
            span = max(hi - lo, 1)
            for i, pc in enumerate(pieces):
                slot_sched[min(lo + (i * span) // n, NSLOT - 1)].append(pc)

        # head: the critical chain is five DMAs (wq0, x^T chunk-0 hi, wk0,
        # xe^T chunk-0 half 0, x^T chunk-0 lo) plus two projection pieces.
        # x chunk 0 ships hi-part first so the two x8-terms of the q
        # projection overlap the xe half DMA.
        kx, kxe, kxe1 = ("x", 0), ("xe", 0), ("xe", 1)
        load_w("wq", wq, 0)()
        chunk_load_part(xpk, kx, 0, 0)()
        # the six hi-term q matmuls are emitted before the remaining head
        # loads so they depend only on wq + x0-hi and start mid-ramp
        kq_piece("wq", qT, kx, 0, 0, part=0)()
        load_w("wk", wk, 0)()
        xe0_half_load(0)()
        chunk_load_part(xpk, kx, 0, 1)()
        load_w("wv", wv, 0)()
        xe0_half_load(1)()
        chunk_load(xepk, kxe1, 1)()
        kq_piece("wq", qT, kx, 0, 0, part=1)()
        # the first scores need only s-tiles 0-1, which ride the first
        # half DMA; the s23 halves follow in the head stream
        kq_piece("wk", kT, kxe, 0, 0, s4=(0, 2))()
        v_piece(kxe, 0, 0, 0, 2)()
        kq_piece("wk", kT, kxe, 0, 0, s4=(2, 4))()
        v_piece(kxe, 0, 0, 2, 4)()

        # ramp era: chunk 1 and the dsl=1 halves in first-use order
        slot_sched[0] += [load_w("wq", wq, 1), load_w("wk", wk, 1),
                          load_w("wv", wv, 1),
                          kq_piece("wk", kT, kxe1, 1, 0, part=0)]
        slot_sched[1] += [kq_piece("wk", kT, kxe1, 1, 0, part=1),
                          v_piece(kxe1, 1, 0, part=0)]
        slot_sched[2] += [v_piece(kxe1, 1, 0, part=1)]
        slot_sched[3] += [kq_piece("wk", kT, kxe, 0, 1, part=0),
                          kq_piece("wk", kT, kxe, 0, 1, part=1)]
        slot_sched[4] += [kq_piece("wq", qT, kx, 0, 1, part=0),
                          kq_piece("wq", qT, kx, 0, 1, part=1),
                          v_piece(kxe, 0, 1, part=0)]
        slot_sched[5] += [v_piece(kxe, 0, 1, part=1),
                          kq_piece("wk", kT, kxe1, 1, 1, part=0)]
        slot_sched[6] += [kq_piece("wk", kT, kxe1, 1, 1, part=1),
                          v_piece(kxe1, 1, 1, part=0)]
        slot_sched[7] += [v_piece(kxe1, 1, 1, part=1)]

        # steady chunks: q(x-chunk tb) is first used at slot tb*16; the
        # pass-1/2 xe chunks at slots 64 / 96. Pieces spread over WIDE
        # windows so per-slot PE load stays near-uniform (narrow bursts
        # make those windows PE-bound while others sit exp-bound).
        qsh = int(os.environ.get("K_QSH", "2"))
        pfd = int(os.environ.get("K_PFD", "3"))
        for tb, (lo_, use) in ((1, (5 + qsh, 16)), (2, (18 + qsh, 32)),
                               (3, (34 + qsh, 48))):
            key = ("x", tb)
            spread([chunk_load(xpk, key, tb)], lo_ - pfd, lo_ - pfd + 1)
            spread([kq_piece("wq", qT, key, tb, d, part=pt)
                    for d in range(NSW) for pt in (0, 1)],
                   lo_, use - 1)
        xe2lo = int(os.environ.get("K_XE2LO", "36"))
        xe3lo = int(os.environ.get("K_XE3LO", "52"))
        xe_sched = (((2, (xe2lo, 64)), (3, (xe3lo, 68))) if TWO_PASS
                    else ((2, (36, 64)), (3, (66, 96))))
        for c, (lo_, use) in xe_sched:
            key = ("xe", c)
            spread([chunk_load(xepk, key, c)], lo_ - pfd + 1, lo_ - pfd + 2)
            spread([kq_piece("wk", kT, key, c, 0, part=0),
                    kq_piece("wk", kT, key, c, 0, part=1),
                    v_piece(key, c, 0, part=0),
                    v_piece(key, c, 0, part=1),
                    kq_piece("wk", kT, key, c, 1, part=0),
                    kq_piece("wk", kT, key, c, 1, part=1),
                    v_piece(key, c, 1, part=0),
                    v_piece(key, c, 1, part=1)],
                   lo_, use - 1)

        # Interleave of the 256 per-head exp half-slots between ScalarE
        # (native exp) and VectorE (custom poly op): greedy virtual-time by
        # default (each exp to the engine with less accumulated work, so the
        # split adapts per era), or fixed per-era Bresenham shares when
        # FRONT_SHARE >= 0.
        bres = {"n": 0, "a": 0}

        def emit_exp(p_tile, sc_tile, cur_slot, act_ns=ACT_EXP_NS,
                     dve_ns=DVE_EXP_NS):
            if FRONT_SHARE >= 0:
                share = FRONT_SHARE if cur_slot < 64 else BACK_SHARE
                bres["n"] += 1
                on_act = bres["a"] + 1 <= share * bres["n"]
                if on_act:
                    bres["a"] += 1
            else:
                on_act = vtime["act"] + act_ns <= vtime["dve"] + dve_ns
            if on_act:
                vtime["act"] += act_ns
                nc.scalar.activation(out=p_tile, in_=sc_tile, func=AF.Exp,
                                     scale=SCALE_Q)
            else:
                vtime["dve"] += dve_ns
                nc.vector._custom_dve(_EXP_OP, out=p_tile, in0=sc_tile,
                                      s0=SCALE_Q / 8.0, s1=0.5)

        # ------------------------------------------------------------------
        # attention: passes over uneven st ranges. Pass 0 covers xe chunks
        # 0-1 (built during the DMA-bound ramp); chunks 2 and 3 are only
        # pulled in at slots 64 / 96, so their projection pieces land in the
        # otherwise PE-lighter second half.
        # ------------------------------------------------------------------
        slot = 0
        passes = ([(0, 8), (8, 16)] if TWO_PASS
                  else [(0, 8), (8, 12), (12, 16)])
        glist = [(lo, hi, tb, sw) for (lo, hi) in passes
                 for tb in range(NTB) for sw in range(NSW)]
        lastv = {}
        seen = set()
        for gi, (lo, hi, tb, sw) in enumerate(glist):
            lastv[(tb, sw)] = gi
        for gi, (lo, hi, tb, sw) in enumerate(glist):
            last_g = gi == len(glist) - 1
            if last_g:
                # pre-charge the tail's DVE-only work so the balancer
                # leans the last exps onto ScalarE and both streams
                # drain together
                vtime["dve"] += TAIL_PRE_NS
            accs = [pacc.tile([P, 2, 2, D + 1], F32, tag="acc",
                              name=f"acc{a}") for a in range(2)]
            if last_g and TAIL_PRELOAD:
                # last group: preload its psum accumulators with the
                # pass-0 partials (copies run in earlier slack), PV
                # accumulates on top, and finalize reads PSUM directly —
                # the serial tail merges disappear
                for a in range(2):
                    src = acc_sb[:, tb * 4 + 2 * a: tb * 4 + 2 * a + 2,
                                 2 * sw:2 * sw + 2, :]
                    if a == 0:
                        vtime["act"] += DVE_MERGE_NS
                        nc.scalar.copy(out=accs[a], in_=src)
                    else:
                        vtime["dve"] += DVE_MERGE_NS
                        nc.vector.tensor_copy(out=accs[a], in_=src)
                first_pv = [False, False]
            else:
                first_pv = [True, True]

            def do_merge(a):
                dst = acc_sb[:, tb * 4 + 2 * a: tb * 4 + 2 * a + 2,
                             2 * sw:2 * sw + 2, :]
                # (vtime for merges is pre-charged 2 slots ahead in the
                # slot loop so the balancer has lookahead)
                if (tb, sw) not in seen:
                    if MERGE_COPY_ACT:
                        nc.scalar.copy(out=dst, in_=accs[a])
                    else:
                        nc.vector.tensor_copy(out=dst, in_=accs[a])
                else:
                    nc.vector.tensor_add(dst, accs[a], dst)

            def pv_batch(st, tail=False):
                ent = pend.pop(0)
                if ent[0] == "pair":
                    p2 = ent[1]
                    pts = (p2[:, 0], p2[:, 1])
                else:
                    pts = (ent[1], ent[2])
                for tt in range(TB // P):
                    a = tt // 2
                    for h2 in range(2):
                        nc.tensor.matmul(
                            accs[a][:, tt % 2, h2, :],
                            pts[h2][:, tt * P:(tt + 1) * P],
                            v_sb[:, st, 2 * sw + h2, :],
                            start=first_pv[a],
                            stop=(st == hi - 1 and tt % 2 == 1
                                  and h2 == 1),
                            skip_group_check=True)
                        first_pv[a] = False
                    # on the very last batch, merge each accumulator the
                    # moment its final PV is in and chase it with that
                    # half's normalize/store chain, pipelining the tail
                    if tail and tt % 2 == 1:
                        if TAIL_PRELOAD:
                            _finalize(nc, fin, acc_sb, out, tb, sw,
                                      half=tt // 2, on_act=True,
                                      psum_acc=accs[tt // 2])
                        else:
                            do_merge(tt // 2)
                            _finalize(nc, fin, acc_sb, out, tb, sw,
                                      half=tt // 2, on_act=True)

            pend = []
            for st in range(lo, hi):
                if st == hi - LOOKAHEAD and gi != len(glist) - 1:
                    # lookahead: charge this group's upcoming merges (and
                    # finalize reciprocals) before the last exps are
                    # assigned, so the engines drain together at the
                    # boundary
                    if (tb, sw) not in seen and MERGE_COPY_ACT:
                        vtime["act"] += 2 * DVE_MERGE_NS
                    else:
                        vtime["dve"] += 2 * DVE_MERGE_NS
                    if lastv[(tb, sw)] == gi:
                        vtime["dve"] += 2 * DVE_RECIP_NS
                # In the chunk-0/1 era, pieces PRODUCE the kT/qT/v this
                # very slot consumes, so they must precede it in the
                # in-order engine streams. In steady state pieces feed
                # later slots only and are emitted between the exp and the
                # trailing PV batch.
                if slot < 8:
                    for pc in slot_sched[slot]:
                        pc()
                if slot >= PAIR_START:
                    # late era: the projection psum pool is dead, so its
                    # banks widen the score ring. Two modes: 6 one-bank
                    # split tiles (3 slots of PE lookahead), or 3 two-bank
                    # paired tiles with ONE exp instruction per slot
                    # (less per-instruction overhead on the exp engines,
                    # which saturate in this era).
                    if psc2_box[0] is None:
                        front_psum.close()
                        psc2_box[0] = ctx.enter_context(
                            tc.tile_pool(name="psc2",
                                         bufs=(3 if PAIR_MODE else 6),
                                         space="PSUM"))
                    if PAIR_MODE:
                        sc2 = psc2_box[0].tile([P, 2, TB], F32, tag="scp",
                                               name="scp")
                        for h2 in range(2):
                            nc.tensor.matmul(
                                sc2[:, h2, :],
                                kT[h2 * D:(h2 + 1) * D, sw,
                                   st * P:(st + 1) * P],
                                qT[h2 * D:(h2 + 1) * D, sw,
                                   tb * TB:(tb + 1) * TB],
                                start=True, stop=True)
                        p2 = ppool_sb.tile([P, 2, TB], BF16, tag="p2",
                                           name="p2", bufs=4)
                        emit_exp(p2, sc2, slot, ACT_PAIR_NS, DVE_PAIR_NS)
                        pend.append(("pair", p2))
                    else:
                        ptiles = []
                        for h2 in range(2):
                            sc = psc2_box[0].tile([P, TB], F32, tag="sc2",
                                                  name="sc2")
                            nc.tensor.matmul(
                                sc,
                                kT[h2 * D:(h2 + 1) * D, sw,
                                   st * P:(st + 1) * P],
                                qT[h2 * D:(h2 + 1) * D, sw,
                                   tb * TB:(tb + 1) * TB],
                                start=True, stop=True)
                            p = ppool_sb.tile([P, TB], BF16, tag="p",
                                              name="p")
                            emit_exp(p, sc, slot)
                            ptiles.append(p)
                        pend.append(("split", ptiles[0], ptiles[1]))
                else:
                    ptiles = []
                    for h2 in range(2):
                        sc = psc.tile([P, TB], F32, tag="sc", name="sc")
                        nc.tensor.matmul(
                            sc,
                            kT[h2 * D:(h2 + 1) * D, sw, st * P:(st + 1) * P],
                            qT[h2 * D:(h2 + 1) * D, sw,
                               tb * TB:(tb + 1) * TB],
                            start=True, stop=True)
                        p = ppool_sb.tile([P, TB], BF16, tag="p", name="p")
                        emit_exp(p, sc, slot)
                        ptiles.append(p)
                    pend.append(("split", ptiles[0], ptiles[1]))
                if slot >= 8:
                    for pc in slot_sched[slot]:
                        pc()
                # forget old virtual-clock skew: in PE-bound eras both exp
                # engines idle together, so only ~1 slot of accumulated
                # imbalance is real
                m = max(vtime["act"], vtime["dve"]) - CLAMP_NS
                vtime["act"] = max(vtime["act"], m)
                vtime["dve"] = max(vtime["dve"], m)
                # PV batches trail PEND_TRAIL slots behind the exp stream
                if st - lo >= PEND_TRAIL:
                    pv_batch(st - PEND_TRAIL)
                if st == hi - 1:
                    for k in range(max(lo, hi - PEND_TRAIL), hi):
                        pv_batch(k, tail=(gi == len(glist) - 1
                                          and k == hi - 1))
                slot += 1
            # merge psum partials into the SBUF accumulator
            if gi != len(glist) - 1:
                for a in range(2):
                    do_merge(a)
            seen.add((tb, sw))
            if lastv[(tb, sw)] == gi and gi != len(glist) - 1:
                _finalize(nc, fin, acc_sb, out, tb, sw)
        if psc2_box[0] is None:
            front_psum.close()


def _finalize(nc, fin, acc_sb, out, tb, sw, half=None, on_act=False,
              psum_acc=None):
    """Normalize the finished heads of t-block tb and store. Two DMAs
    (2 t-tiles each) so the second store's DGE setup hides under the
    first's transfer; half=0/1 emits one accumulator-half's chain only
    (used to pipeline the very last group's tail). Reciprocal on VectorE,
    normalize-muls on Pool (on ScalarE for the tail, where it sits idle
    and Pool's serial launch overhead would stretch the ending)."""
    halves = (0, 1) if half is None else (half,)
    for h in halves:
        rcp = fin.tile([P, 2, 2], F32, tag="rcp", name="rcp")
        nc.vector.reciprocal(
            out=rcp,
            in_=(psum_acc[:, :, :, D] if psum_acc is not None else
                 acc_sb[:, tb * 4 + 2 * h:tb * 4 + 2 * h + 2,
                        2 * sw:2 * sw + 2, D]))
        ostage = fin.tile([P, 2, 2 * D], F32, tag="ost", name="ostage")
        for mi, (i, h2) in enumerate((i, h2) for i in range(2)
                                     for h2 in range(2)):
            tt4 = 2 * h + i
            dst = ostage[:, i, h2 * D:(h2 + 1) * D]
            src = (psum_acc[:, i, h2, 0:D] if psum_acc is not None else
                   acc_sb[:, tb * 4 + tt4, 2 * sw + h2, 0:D])
            scl = rcp[:, i, h2:h2 + 1]
            if psum_acc is not None:
                # psum source: only ScalarE/VectorE can read it
                if mi % 2 == 0:
                    nc.scalar.mul(out=dst, in_=src, mul=scl)
                else:
                    nc.vector.tensor_scalar_mul(out=dst, in0=src,
                                                scalar1=scl)
            elif not on_act:
                nc.gpsimd.tensor_scalar_mul(out=dst, in0=src, scalar1=scl)
            elif h == 1:
                # tail: the last store gates the kernel end; half-1's muls
                # run on the by-then-idle VectorE (94ns each) except one
                # on ScalarE for overlap
                if mi == 0:
                    nc.scalar.mul(out=dst, in_=src, mul=scl)
                else:
                    nc.vector.tensor_scalar_mul(out=dst, in0=src,
                                                scalar1=scl)
            elif mi == 0:
                # spread half-0's muls across ACT/Pool/DVE
                nc.scalar.mul(out=dst, in_=src, mul=scl)
            elif mi == 2:
                nc.vector.tensor_scalar_mul(out=dst, in0=src, scalar1=scl)
            else:
                nc.gpsimd.tensor_scalar_mul(out=dst, in0=src, scalar1=scl)
        t0 = (tb * 4 + h * 2) * P
        q = nc.scalar if (on_act and h == 0) else nc.sync
        q.dma_start(
            out=out[t0:t0 + 2 * P,
                    sw * 2 * D:(sw + 1) * 2 * D].rearrange(
                        "(tt p) c -> p tt c", p=P),
            in_=ostage)


def build_program():
    nc = bacc.Bacc("TRN2", target_bir_lowering=False, debug=False,
                   num_devices=N_CORES)

    # Host ships x/x_enc as packed hi/lo fp8 DR layouts (same bytes as
    # bf16) and W*32 hi/lo fp8; all transposes ride the DMA crossbar.
    xpk = nc.dram_tensor("x", [NCH, 2, P, NJ, 2, 512], FP8,
                         kind="ExternalInput").ap()
    xepk = nc.dram_tensor("xe", [NCH, 2, P, NJ, 2, 512], FP8,
                          kind="ExternalInput").ap()
    xe0 = nc.dram_tensor("xe0", [2, 2, P, NJ, 2, 256], FP8,
                         kind="ExternalInput").ap()
    wq = nc.dram_tensor("wq", [NSW, 2, P, NJ, 2, P], FP8,
                        kind="ExternalInput").ap()
    wk = nc.dram_tensor("wk", [NSW, 2, P, NJ, 2, P], FP8,
                        kind="ExternalInput").ap()
    wv = nc.dram_tensor("wv", [NSW, 2, P, NJ, 2, P], FP8,
                        kind="ExternalInput").ap()
    out = nc.dram_tensor("out", [T, DCORE], F32, kind="ExternalOutput").ap()

    with tile.TileContext(nc) as tc:
        _build_body(nc, tc, xpk, xepk, xe0, wq, wk, wv, out)
    nc.compile()
    return nc


_NC_CACHE = None


def _get_program():
    global _NC_CACHE
    if _NC_CACHE is None:
        _NC_CACHE = build_program()
    return _NC_CACHE


_F8 = ml_dtypes.float8_e4m3


def _split8(a):
    hi = a.astype(_F8)
    lo = (a - hi.astype(np.float32)).astype(_F8)
    return hi, lo


def _pack_x(xT):
    """[C, T] f32 -> [NCH, 2, P, NJ, 2, 512] fp8 hi/lo, c = j*256+i*128+k."""
    o = np.empty((NCH, 2, P, NJ, 2, 512), dtype=_F8)
    for cch in range(NCH):
        xc = xT[:, cch * 512:(cch + 1) * 512]
        hi, lo = _split8(xc)
        for t, a in ((0, hi), (1, lo)):
            o[cch, t] = a.reshape(NJ, 2, P, 512).transpose(2, 0, 1, 3)
    return np.ascontiguousarray(o)


def _pack_xe0(xeT):
    """First 512 cols of xe^T -> [2 half, 2, P, NJ, 2, 256] fp8."""
    o = np.empty((2, 2, P, NJ, 2, 256), dtype=_F8)
    for h in range(2):
        xc = xeT[:, h * 256:(h + 1) * 256]
        hi, lo = _split8(xc)
        for t, a in ((0, hi), (1, lo)):
            o[h, t] = a.reshape(NJ, 2, P, 256).transpose(2, 0, 1, 3)
    return np.ascontiguousarray(o)


def _pack_w(w):
    """[1024, 256] f32 -> [NSW, 2, P, NJ, 2, P] fp8 of W*32 hi/lo."""
    w = w * WSCALE
    o = np.empty((NSW, 2, P, NJ, 2, P), dtype=_F8)
    for dsl in range(NSW):
        wd = w.reshape(C, NSW, P)[:, dsl, :]          # cols = dsl*128 + d
        hi, lo = _split8(wd)
        for t, a in ((0, hi), (1, lo)):
            o[dsl, t] = a.reshape(NJ, 2, P, P).transpose(2, 0, 1, 3)
    return np.ascontiguousarray(o)


def kernel(x_enc, x, Wk, Wq, Wv):
    x_enc = np.asarray(x_enc, dtype=np.float32)
    x = np.asarray(x, dtype=np.float32)
    Wk = np.asarray(Wk, dtype=np.float32)
    Wq = np.asarray(Wq, dtype=np.float32)
    Wv = np.asarray(Wv, dtype=np.float32)

    nc = _get_program()
    in_maps = []
    for core in range(N_CORES):
        b, hg = divmod(core, N_CORES // B)
        csl = slice(hg * DCORE, (hg + 1) * DCORE)
        xT = np.ascontiguousarray(x[b].T)
        xeT = np.ascontiguousarray(x_enc[b].T)
        in_maps.append({
            "x": _pack_x(xT),
            "xe": _pack_x(xeT),
            "xe0": _pack_xe0(xeT),
            "wq": _pack_w(Wq[:, csl]),
            "wk": _pack_w(Wk[:, csl]),
            "wv": _pack_w(Wv[:, csl]),
        })
    res = run_bass_kernel_spmd(nc, in_maps, list(range(N_CORES)))

    full = np.empty((B, T, H, D), dtype=np.float32)
    for core in range(N_CORES):
        b, hg = divmod(core, N_CORES // B)
        o = res.results[core]["out"].reshape(T, HG, D)
        full[b, :, hg * HG:(hg + 1) * HG, :] = o
    return full
